# revision 6
# baseline (speedup 1.0000x reference)
"""Co-attention head kernel for 8 Trainium2 NeuronCores.

Reference computation (H=4096, heads=4, d=1024, N=1024):
    q/k/v[h] = node1|node2 @ W{q,k,v}[h] + b        ([N, d] per head)
    r[h]     = (q[h] @ k[h]^T * 1/sqrt(d)) .* v[h]  (elementwise, N==d)
    out      = LayerNorm(concat_h r[h])             ([N, 4096])

Sharding: 8 cores = 4 heads x 2 halves. Core c=(h=c//2, s=c%2):
  - computes qT/kT = W^T @ actT for its 512-wide f-slice of head h
    (weights pre-sliced+scaled on host; activations pre-transposed on host;
    for odd cores n1t's column halves are swapped so that LOCAL score rows
    0:511 are always the core's own rows and 512:1023 the pair's rows —
    keeps the SPMD program symmetric)
  - partial scores over its f-half; own-row partials stay in SBUF, pair-row
    partials go to the HBM-pair neighbor (c^1) via remote_dma_broadcast
    SBUF->SBUF (no ncfw collective, no DRAM bounce); pair adds partials
  - v[n-own, :] via stationary host-sliced "n2v" blocks + moving full Wv
  - r = scores .* v; LayerNorm stats reduced over the quad {c, c^2, c^4, c^6}
    hierarchically: round 1 exchange+add with c^2 (same die), round 2 with
    the other die (value-identical on c^4/c^6 after round 1, so the
    known cross-die tpb^2 routing flip of this ucode cannot corrupt it)
  - writes out block [512, 1024]; host assembles the [1024, 4096] output.

All matmuls bf16 (same PE rate as f32r, half the HBM traffic; rel err ~3e-3
vs 2e-2 budget). Remote-sem waits are appended to instructions' sync_info
AFTER Tile scheduling (the single-core scheduling sim would deadlock on
cross-core sems); cross-rep buffer reuse is guarded by nodata ack
broadcasts with per-rep thresholds.
"""

from contextlib import ExitStack

import numpy as np

import concourse.bass as bass
import concourse.tile as tile
from concourse import bacc, mybir
from concourse.bass_utils import run_bass_kernel_spmd
from concourse.tile_rust import add_dep_helper

F32 = mybir.dt.float32
BF16 = mybir.dt.bfloat16

H_DIM = 4096
N_HEADS = 4
D_HEAD = 1024
N = 1024
LN_EPS = 1e-5
N_CORES = 8
SCALE = 1.0 / 32.0  # 1/sqrt(D_HEAD)

K_TILES = H_DIM // 128  # 32
KB = 4  # k-tiles per stream DMA
WIRE = "bf16"

ALU = mybir.AluOpType
ACT_FN = mybir.ActivationFunctionType

# local-sem increments per rep: 9 broadcast-ish preps x 16
LOC_PER_REP = 9 * 16


def _bcast_ap(ap: bass.AP, parts: int = 128) -> bass.AP:
    """[n] DRAM vector viewed as [parts, n] with 0-stride partitions."""
    return bass.AP(tensor=ap.tensor, offset=ap.offset, ap=[[0, parts], *ap.ap])


def build_program(reps: int = 1, wire: str = WIRE):
    WD = BF16 if wire == "bf16" else mybir.dt.float32r
    nc = bacc.Bacc("TRN2", target_bir_lowering=False, debug=False, num_devices=N_CORES)

    n1t = nc.dram_tensor("n1t", [H_DIM, N], WD, kind="ExternalInput").ap()
    n2t = nc.dram_tensor("n2t", [H_DIM, N], WD, kind="ExternalInput").ap()
    n2v = nc.dram_tensor("n2v", [H_DIM, 512], WD, kind="ExternalInput").ap()
    wq = nc.dram_tensor("wq", [H_DIM, 512], WD, kind="ExternalInput").ap()
    wk = nc.dram_tensor("wk", [H_DIM, 512], WD, kind="ExternalInput").ap()
    wv = nc.dram_tensor("wv", [H_DIM, D_HEAD], WD, kind="ExternalInput").ap()
    bq = nc.dram_tensor("bq", [512], F32, kind="ExternalInput").ap()
    bk = nc.dram_tensor("bk", [512], F32, kind="ExternalInput").ap()
    bv = nc.dram_tensor("bv", [D_HEAD], F32, kind="ExternalInput").ap()
    gam = nc.dram_tensor("gam", [D_HEAD], F32, kind="ExternalInput").ap()
    bet = nc.dram_tensor("bet", [D_HEAD], F32, kind="ExternalInput").ap()
    out = nc.dram_tensor("out", [512, N], F32, kind="ExternalOutput").ap()

    n1t_2 = n1t.rearrange("(a p) n -> p a n", p=128)
    n2t_2 = n2t.rearrange("(a p) n -> p a n", p=128)
    n2v_2 = n2v.rearrange("(a p) n -> p a n", p=128)
    wq_2 = wq.rearrange("(a p) f -> p a f", p=128)
    wk_2 = wk.rearrange("(a p) f -> p a f", p=128)
    wv_2 = wv.rearrange("(a p) f -> p a f", p=128)

    sc_dat = nc.alloc_semaphore("sc_dat")
    sc_ack = nc.alloc_semaphore("sc_ack")
    q2_dat = nc.alloc_semaphore("q2_dat")
    q2_ack = nc.alloc_semaphore("q2_ack")
    q4_dat = nc.alloc_semaphore("q4_dat")
    q4_ack = nc.alloc_semaphore("q4_ack")
    loc = nc.alloc_semaphore("loc")

    D1 = [(0, 1)] * 8
    D2 = [(0, 2)] * 8
    D4 = [None, None, None, None, (0, 4), (0, 4), (0, 4), (0, 4)]

    # (instruction, sem, threshold) to encode after Tile scheduling
    post_waits = []

    def want_wait(binst, sem, val):
        if val > 0:
            post_waits.append((binst, sem, val))

    with tile.TileContext(nc) as tc, ExitStack() as ctx:
        singles = ctx.enter_context(tc.tile_pool(name="singles", bufs=1))
        streams = ctx.enter_context(tc.tile_pool(name="streams", bufs=2))
        resident = ctx.enter_context(tc.tile_pool(name="resident", bufs=1))
        ps = ctx.enter_context(tc.tile_pool(name="ps", bufs=1, space="PSUM"))
        comm = ctx.enter_context(tc.tile_pool(name="comm", bufs=1))
        fin = ctx.enter_context(tc.tile_pool(name="fin", bufs=1))

        # ---- constants (loaded once) ----
        bq_sb = singles.tile([128, 4], F32)
        nc.sync.dma_start(out=bq_sb, in_=bq.rearrange("(b p) -> p b", p=128))
        bk_sb = singles.tile([128, 4], F32)
        nc.sync.dma_start(out=bk_sb, in_=bk.rearrange("(b p) -> p b", p=128))
        bv_b = singles.tile([128, D_HEAD], F32)
        nc.sync.dma_start(out=bv_b, in_=_bcast_ap(bv))
        gam_b = singles.tile([128, D_HEAD], F32)
        nc.sync.dma_start(out=gam_b, in_=_bcast_ap(gam))
        bet_b = singles.tile([128, D_HEAD], F32)
        nc.sync.dma_start(out=bet_b, in_=_bcast_ap(bet))
        eps_sb = singles.tile([128, 1], F32)
        nc.vector.memset(eps_sb, LN_EPS)

        gp_chain = [None]  # chain all gpsimd desc/trigger instrs across reps

        def chain(binst):
            if gp_chain[0] is not None:
                add_dep_helper(binst.ins, gp_chain[0].ins, reason="swdge order")
            gp_chain[0] = binst

        def emit_rep(R):
            loc_free = LOC_PER_REP * R  # all of rep R-1's sends have left
            qT = [
                resident.tile([128, N], WD, name=f"qT{f}", tag=f"qT{f}")
                for f in range(4)
            ]
            kT = [
                resident.tile([128, N], WD, name=f"kT{f}", tag=f"kT{f}")
                for f in range(4)
            ]
            v_sb = [
                resident.tile([128, D_HEAD], F32, name=f"v{t}", tag=f"v{t}")
                for t in range(4)
            ]
            sc_own = [
                comm.tile([128, N], F32, name=f"sco{t}", tag=f"sco{t}")
                for t in range(4)
            ]
            sc_send = comm.tile([128, 4 * N], F32, name="sc_send", tag="sc_send")
            sc_recv = comm.tile([128, 4 * N], F32, name="sc_recv", tag="sc_recv")
            st = comm.tile([128, 4, 2], F32, name="st", tag="st")
            st_r2 = comm.tile([128, 4, 2], F32, name="st_r2", tag="st_r2")
            st12 = comm.tile([128, 4, 2], F32, name="st12", tag="st12")
            st_r4 = comm.tile([128, 4, 2], F32, name="st_r4", tag="st_r4")
            tot = comm.tile([128, 4, 2], F32, name="tot", tag="tot")

            def projection(act2, w2, bias_fn):
                psums = [
                    [
                        ps.tile([128, 512], F32, name=f"pp{f}_{j}", tag=f"pp{f}_{j}")
                        for j in range(2)
                    ]
                    for f in range(4)
                ]
                for k4 in range(K_TILES // KB):
                    a_t = streams.tile([128, KB, N], WD, name="a_t", tag="a_t")
                    w_t = streams.tile([128, KB, 512], WD, name="w_t", tag="w_t")
                    if k4 == 0:
                        # split the first chunk's DMA so the PE can start on
                        # sub-chunk 0 ~3 us earlier
                        nc.sync.dma_start(out=a_t[:, 0:1, :], in_=act2[:, 0:1, :])
                        nc.sync.dma_start(out=a_t[:, 1:KB, :], in_=act2[:, 1:KB, :])
                        nc.scalar.dma_start(out=w_t[:, 0:1, :], in_=w2[:, 0:1, :])
                        nc.scalar.dma_start(out=w_t[:, 1:KB, :], in_=w2[:, 1:KB, :])
                    else:
                        nc.sync.dma_start(
                            out=a_t, in_=act2[:, KB * k4 : KB * k4 + KB, :]
                        )
                        nc.scalar.dma_start(
                            out=w_t, in_=w2[:, KB * k4 : KB * k4 + KB, :]
                        )
                    for a in range(KB):
                        first = k4 == 0 and a == 0
                        last = k4 == K_TILES // KB - 1 and a == KB - 1
                        for f in range(4):
                            for j in range(2):
                                nc.tensor.matmul(
                                    psums[f][j][:],
                                    w_t[:, a, f * 128 : (f + 1) * 128],
                                    a_t[:, a, j * 512 : (j + 1) * 512],
                                    start=first,
                                    stop=last,
                                )
                for f in range(4):
                    for j in range(2):
                        bias_fn(f, j, psums[f][j])

            # ---- Q phase ----
            projection(
                n1t_2,
                wq_2,
                lambda f, j, p: nc.vector.tensor_scalar(
                    out=qT[f][:, j * 512 : (j + 1) * 512],
                    in0=p[:],
                    scalar1=bq_sb[:, f : f + 1],
                    scalar2=None,
                    op0=ALU.add,
                ),
            )
            # ---- K phase ----
            projection(
                n2t_2,
                wk_2,
                lambda f, j, p: nc.vector.tensor_scalar(
                    out=kT[f][:, j * 512 : (j + 1) * 512],
                    in0=p[:],
                    scalar1=bk_sb[:, f : f + 1],
                    scalar2=None,
                    op0=ALU.add,
                ),
            )

            # ---- scores: pair rows (local nb 4..7) first -> sc_send -> rdma;
            #      own rows (nb 0..3) -> sc_own ----
            for nb in range(4, 8):
                t = nb - 4
                for mh in range(2):
                    sc_ps = ps.tile(
                        [128, 512], F32, name=f"sps{nb}_{mh}", tag=f"pp{nb % 4}_{mh}"
                    )
                    for ft in range(4):
                        nc.tensor.matmul(
                            sc_ps[:],
                            qT[ft][:, nb * 128 : (nb + 1) * 128],
                            kT[ft][:, mh * 512 : (mh + 1) * 512],
                            start=(ft == 0),
                            stop=(ft == 3),
                        )
                    cp = nc.vector.tensor_copy(
                        out=sc_send[:, t * N + mh * 512 : t * N + (mh + 1) * 512],
                        in_=sc_ps[:],
                    )
                    want_wait(cp, loc, loc_free)
                b = nc.gpsimd.remote_dma_broadcast(
                    sc_recv[:, t * N : (t + 1) * N],
                    sc_send[:, t * N : (t + 1) * N],
                    sc_dat,
                    loc,
                    rdests=D1,
                )
                chain(b)
            t1 = nc.gpsimd.trigger_dma(count=None)
            chain(t1)
            want_wait(t1, sc_ack, 16 * R)

            for nb in range(4):
                for mh in range(2):
                    sc_ps = ps.tile(
                        [128, 512], F32, name=f"sps{nb}_{mh}", tag=f"pp{nb % 4}_{mh}"
                    )
                    for ft in range(4):
                        nc.tensor.matmul(
                            sc_ps[:],
                            qT[ft][:, nb * 128 : (nb + 1) * 128],
                            kT[ft][:, mh * 512 : (mh + 1) * 512],
                            start=(ft == 0),
                            stop=(ft == 3),
                        )
                    nc.vector.tensor_copy(
                        out=sc_own[nb][:, mh * 512 : (mh + 1) * 512], in_=sc_ps[:]
                    )

            # ---- V phase: stationary n2v blocks, moving full wv ----
            vps = [
                [
                    ps.tile([128, 512], F32, name=f"vp{t}_{j}", tag=f"pp{t}_{j}")
                    for j in range(2)
                ]
                for t in range(4)
            ]
            for k4 in range(K_TILES // KB):
                nv_t = streams.tile([128, KB, 512], WD, name="nv_t", tag="nv_t")
                nc.scalar.dma_start(out=nv_t, in_=n2v_2[:, KB * k4 : KB * k4 + KB, :])
                wv_t = streams.tile([128, KB, D_HEAD], WD, name="wv_t", tag="wv_t")
                nc.sync.dma_start(out=wv_t, in_=wv_2[:, KB * k4 : KB * k4 + KB, :])
                for a in range(KB):
                    first = k4 == 0 and a == 0
                    last = k4 == K_TILES // KB - 1 and a == KB - 1
                    for t in range(4):
                        for j in range(2):
                            nc.tensor.matmul(
                                vps[t][j][:],
                                nv_t[:, a, t * 128 : (t + 1) * 128],
                                wv_t[:, a, j * 512 : (j + 1) * 512],
                                start=first,
                                stop=last,
                            )

            # ---- r_pre = own + recv partials (overlaps V matmuls) ----
            r_pre = [
                fin.tile([128, N], F32, name=f"rp{t}", tag=f"rp{t}") for t in range(4)
            ]
            adds = []
            for t in range(4):
                ad = nc.vector.tensor_add(
                    out=r_pre[t][:],
                    in0=sc_own[t][:],
                    in1=sc_recv[:, t * N : (t + 1) * N],
                )
                want_wait(ad, sc_dat, 64 * (R + 1))
                adds.append(ad)

            for t in range(4):
                for j in range(2):
                    nc.vector.tensor_add(
                        out=v_sb[t][:, j * 512 : (j + 1) * 512],
                        in0=vps[t][j][:],
                        in1=bv_b[:, j * 512 : (j + 1) * 512],
                    )

            # ---- r = r_pre .* v; LN stat partials ----
            sq_scr = fin.tile([128, N], F32, name="sq_scr", tag="sq_scr")
            st_writers = []
            for t in range(4):
                nc.vector.tensor_mul(out=r_pre[t][:], in0=r_pre[t][:], in1=v_sb[t][:])
                rd = nc.vector.tensor_reduce(
                    out=st[:, t, 0:1], in_=r_pre[t][:],
                    axis=mybir.AxisListType.X, op=ALU.add,
                )
                sqa = nc.scalar.activation(
                    out=sq_scr[:], in_=r_pre[t][:], func=ACT_FN.Square,
                    accum_out=st[:, t, 1:2],
                )
                want_wait(rd, loc, loc_free)
                want_wait(sqa, loc, loc_free)
                st_writers.append((rd, sqa))

            # ---- stats round 1: exchange with c^2 (same die) ----
            b2 = nc.gpsimd.remote_dma_broadcast(st_r2[:], st[:], q2_dat, loc, rdests=D2)
            chain(b2)
            t2 = nc.gpsimd.trigger_dma(count=None)
            chain(t2)
            want_wait(t2, q2_ack, 16 * R)
            a12 = nc.vector.tensor_add(out=st12[:], in0=st[:], in1=st_r2[:])
            want_wait(a12, q2_dat, 16 * (R + 1))
            want_wait(a12, loc, loc_free)

            # ---- stats round 2: exchange with the other die (flip-immune) ----
            b4 = nc.gpsimd.remote_dma_broadcast(st_r4[:], st12[:], q4_dat, loc, rdests=D4)
            chain(b4)
            t3 = nc.gpsimd.trigger_dma(count=None)
            chain(t3)
            want_wait(t3, q4_ack, 8 * R)
            atot = nc.vector.tensor_add(out=tot[:], in0=st12[:], in1=st_r4[:])
            want_wait(atot, q4_dat, 8 * (R + 1))

            # ---- acks (consumed-my-recv-buffers) for next rep ----
            k1 = nc.gpsimd.remote_sem_update_broadcast(sc_ack, loc, rdests=D1)
            for ad in adds:
                add_dep_helper(k1.ins, ad.ins, reason="sc_recv consumed")
            chain(k1)
            k2 = nc.gpsimd.remote_sem_update_broadcast(q2_ack, loc, rdests=D2)
            add_dep_helper(k2.ins, a12.ins, reason="st_r2 consumed")
            chain(k2)
            k4i = nc.gpsimd.remote_sem_update_broadcast(q4_ack, loc, rdests=D4)
            add_dep_helper(k4i.ins, atot.ins, reason="st_r4 consumed")
            chain(k4i)
            t4 = nc.gpsimd.trigger_dma(count=None)
            chain(t4)

            # ---- normalize + out ----
            inv_h = 1.0 / float(H_DIM)
            for t in range(4):
                mu_t = fin.tile([128, 1], F32, name=f"mu{t}", tag=f"mu{t}")
                nc.vector.tensor_scalar_mul(out=mu_t, in0=tot[:, t, 0:1], scalar1=inv_h)
                msq_t = fin.tile([128, 1], F32, name=f"msq{t}", tag=f"msq{t}")
                nc.vector.tensor_mul(out=msq_t, in0=mu_t, in1=mu_t)
                var_t = fin.tile([128, 1], F32, name=f"var{t}", tag=f"var{t}")
                nc.vector.tensor_scalar(
                    out=var_t,
                    in0=tot[:, t, 1:2],
                    scalar1=inv_h,
                    scalar2=msq_t[:, 0:1],
                    op0=ALU.mult,
                    op1=ALU.subtract,
                )
                nc.scalar.activation(
                    out=var_t, in_=var_t, func=ACT_FN.Sqrt, bias=eps_sb[:], scale=1.0
                )
                nc.vector.reciprocal(out=var_t, in_=var_t)
                o_t = fin.tile([128, N], F32, name="o_t", tag="o_t", bufs=2)[:]
                nc.vector.tensor_scalar(
                    out=o_t,
                    in0=r_pre[t][:],
                    scalar1=mu_t[:, 0:1],
                    scalar2=var_t[:, 0:1],
                    op0=ALU.subtract,
                    op1=ALU.mult,
                )
                nc.vector.tensor_mul(out=o_t, in0=o_t, in1=gam_b[:])
                nc.vector.tensor_add(out=o_t, in0=o_t, in1=bet_b[:])
                nc.sync.dma_start(out=out[t * 128 : (t + 1) * 128, :], in_=o_t)

        for rep in range(reps):
            emit_rep(rep)

    # encode runtime-only cross-core waits (post-scheduling)
    for binst, sem, val in post_waits:
        ins = binst.ins
        si = ins.sync_info
        waits = list(si.on_wait) if si else []
        ups = list(si.on_update) if si else []
        waits.append(
            mybir.SyncWait(
                sync_type="semaphore", id=sem.num, ant_name=sem.name,
                wait_mode="sem-ge-imm", wait_value=val,
            )
        )
        ins.sync_info = mybir.SyncInfo(on_wait=waits, on_update=ups)

    nc.compile()
    return nc


_NC = None


def _get_program():
    global _NC
    if _NC is None:
        _NC = build_program()
    return _NC


def make_in_maps(node1, node2, Wq, bq, Wk, bk, Wv, bv, gamma, beta, wire: str = WIRE):
    import ml_dtypes

    f32 = np.float32
    wd = ml_dtypes.bfloat16 if wire == "bf16" else np.float32
    n1t = np.ascontiguousarray(np.asarray(node1).T, dtype=f32)
    n2t = np.ascontiguousarray(np.asarray(node2).T, dtype=f32)
    n1t_w = n1t.astype(wd)
    n2t_w = n2t.astype(wd)
    # odd cores see node1's row halves swapped -> local score rows 0:511
    # are always the core's own rows (SPMD-symmetric send/keep split)
    n1t_sw = np.ascontiguousarray(
        np.concatenate([n1t_w[:, 512:], n1t_w[:, :512]], axis=1)
    )
    in_maps = []
    for c in range(N_CORES):
        h, s = c // 2, c % 2
        fsl = slice(s * 512, (s + 1) * 512)
        nsl = slice(s * 512, (s + 1) * 512)
        in_maps.append(
            {
                "n1t": n1t_w if s == 0 else n1t_sw,
                "n2t": n2t_w,
                "n2v": np.ascontiguousarray(n2t_w[:, nsl]),
                "wq": np.ascontiguousarray(Wq[h][:, fsl] * SCALE).astype(wd),
                "wk": np.ascontiguousarray(Wk[h][:, fsl]).astype(wd),
                "wv": np.ascontiguousarray(Wv[h]).astype(wd),
                "bq": np.ascontiguousarray(bq[h][fsl] * SCALE, dtype=f32),
                "bk": np.ascontiguousarray(bk[h][fsl], dtype=f32),
                "bv": np.ascontiguousarray(bv[h], dtype=f32),
                "gam": np.ascontiguousarray(gamma[h * 1024 : (h + 1) * 1024], dtype=f32),
                "bet": np.ascontiguousarray(beta[h * 1024 : (h + 1) * 1024], dtype=f32),
            }
        )
    return in_maps


def assemble(results):
    out = np.empty((N, H_DIM), np.float32)
    for c in range(N_CORES):
        h, s = c // 2, c % 2
        out[s * 512 : (s + 1) * 512, h * 1024 : (h + 1) * 1024] = results[c]["out"]
    return out


def kernel(node1, node2, Wq, bq, Wk, bk, Wv, bv, gamma, beta):
    nc = _get_program()
    in_maps = make_in_maps(node1, node2, Wq, bq, Wk, bk, Wv, bv, gamma, beta)
    res = run_bass_kernel_spmd(nc, in_maps, list(range(N_CORES)))
    return assemble(res.results)


# revision 9
# speedup vs baseline: 1.2147x; 1.2147x over previous
"""Co-attention head kernel for 8 Trainium2 NeuronCores.

Reference computation (H=4096, heads=4, d=1024, N=1024):
    q/k/v[h] = node1|node2 @ W{q,k,v}[h] + b        ([N, d] per head)
    r[h]     = (q[h] @ k[h]^T * 1/sqrt(d)) .* v[h]  (elementwise, N==d)
    out      = LayerNorm(concat_h r[h])             ([N, 4096])

Sharding: 8 cores = 4 heads x 2 halves. Core c=(h=c//2, s=c%2):
  - computes qT/kT = W^T @ actT for its 512-wide f-slice of head h
    (weights pre-sliced+scaled on host; activations pre-transposed on host;
    for odd cores n1t's column halves are swapped so that LOCAL score rows
    0:511 are always the core's own rows and 512:1023 the pair's rows —
    keeps the SPMD program symmetric)
  - partial scores over its f-half; own-row partials stay in SBUF, pair-row
    partials go to the HBM-pair neighbor (c^1) via remote_dma_broadcast
    SBUF->SBUF (no ncfw collective, no DRAM bounce); pair adds partials
  - v[n-own, :] via stationary host-sliced "n2v" blocks + moving full Wv
  - r = scores .* v; LayerNorm stats reduced over the quad {c, c^2, c^4, c^6}
    hierarchically: round 1 exchange+add with c^2 (same die), round 2 with
    the other die (value-identical on c^4/c^6 after round 1, so the
    known cross-die tpb^2 routing flip of this ucode cannot corrupt it)
  - writes out block [512, 1024]; host assembles the [1024, 4096] output.

All matmuls bf16 (same PE rate as f32r, half the HBM traffic; rel err ~3e-3
vs 2e-2 budget). Remote-sem waits are appended to instructions' sync_info
AFTER Tile scheduling (the single-core scheduling sim would deadlock on
cross-core sems); cross-rep buffer reuse is guarded by nodata ack
broadcasts with per-rep thresholds.
"""

from contextlib import ExitStack

import numpy as np

import concourse.bass as bass
import concourse.tile as tile
from concourse import bacc, mybir
from concourse.bass_utils import run_bass_kernel_spmd
from concourse.tile_rust import add_dep_helper

F32 = mybir.dt.float32
BF16 = mybir.dt.bfloat16

H_DIM = 4096
N_HEADS = 4
D_HEAD = 1024
N = 1024
LN_EPS = 1e-5
N_CORES = 8
SCALE = 1.0 / 32.0  # 1/sqrt(D_HEAD)

K_TILES = H_DIM // 128  # 32
KB = 4  # k-tiles per stream DMA
WIRE = "bf16"

ALU = mybir.AluOpType
ACT_FN = mybir.ActivationFunctionType

# local-sem increments per rep: 9 broadcast-ish preps x 16
LOC_PER_REP = 9 * 16


def _bcast_ap(ap: bass.AP, parts: int = 128) -> bass.AP:
    """[n] DRAM vector viewed as [parts, n] with 0-stride partitions."""
    return bass.AP(tensor=ap.tensor, offset=ap.offset, ap=[[0, parts], *ap.ap])


def build_program(reps: int = 1, wire: str = WIRE, local_comm: bool = False,
                  stats_ag: bool = False):
    WD = BF16 if wire == "bf16" else mybir.dt.float32r
    nc = bacc.Bacc("TRN2", target_bir_lowering=False, debug=False, num_devices=N_CORES)

    n1t = nc.dram_tensor("n1t", [H_DIM, N], WD, kind="ExternalInput").ap()
    n2t = nc.dram_tensor("n2t", [H_DIM, N], WD, kind="ExternalInput").ap()
    n2v = nc.dram_tensor("n2v", [H_DIM, 512], WD, kind="ExternalInput").ap()
    wq = nc.dram_tensor("wq", [H_DIM, 512], WD, kind="ExternalInput").ap()
    wk = nc.dram_tensor("wk", [H_DIM, 512], WD, kind="ExternalInput").ap()
    wv = nc.dram_tensor("wv", [H_DIM, D_HEAD], WD, kind="ExternalInput").ap()
    bq = nc.dram_tensor("bq", [512], F32, kind="ExternalInput").ap()
    bk = nc.dram_tensor("bk", [512], F32, kind="ExternalInput").ap()
    bv = nc.dram_tensor("bv", [D_HEAD], F32, kind="ExternalInput").ap()
    gam = nc.dram_tensor("gam", [D_HEAD], F32, kind="ExternalInput").ap()
    bet = nc.dram_tensor("bet", [D_HEAD], F32, kind="ExternalInput").ap()
    out = nc.dram_tensor("out", [512, N], F32, kind="ExternalOutput").ap()

    n1t_2 = n1t.rearrange("(a p) n -> p a n", p=128)
    n2t_2 = n2t.rearrange("(a p) n -> p a n", p=128)
    n2v_2 = n2v.rearrange("(a p) n -> p a n", p=128)
    wq_2 = wq.rearrange("(a p) f -> p a f", p=128)
    wk_2 = wk.rearrange("(a p) f -> p a f", p=128)
    wv_2 = wv.rearrange("(a p) f -> p a f", p=128)

    sc_dat = nc.alloc_semaphore("sc_dat")
    sc_ack = nc.alloc_semaphore("sc_ack")
    q2_dat = nc.alloc_semaphore("q2_dat")
    q2_ack = nc.alloc_semaphore("q2_ack")
    q4_dat = nc.alloc_semaphore("q4_dat")
    q4_ack = nc.alloc_semaphore("q4_ack")
    loc = nc.alloc_semaphore("loc")

    D1 = [(0, 1)] * 8
    D2 = [(0, 2)] * 8
    D4 = [None, None, None, None, (0, 4), (0, 4), (0, 4), (0, 4)]

    # (instruction, sem, threshold) to encode after Tile scheduling
    post_waits = []

    def want_wait(binst, sem, val):
        if val > 0 and not local_comm:
            post_waits.append((binst, sem, val))

    with tile.TileContext(nc) as tc, ExitStack() as ctx:
        singles = ctx.enter_context(tc.tile_pool(name="singles", bufs=1))
        streams = ctx.enter_context(tc.tile_pool(name="streams", bufs=2))
        resident = ctx.enter_context(tc.tile_pool(name="resident", bufs=1))
        ps = ctx.enter_context(tc.tile_pool(name="ps", bufs=1, space="PSUM"))
        comm = ctx.enter_context(tc.tile_pool(name="comm", bufs=1))
        fin = ctx.enter_context(tc.tile_pool(name="fin", bufs=1))
        dram = ctx.enter_context(tc.tile_pool(name="dram", bufs=1, space="DRAM"))

        # ---- constants (loaded once) ----
        bq_sb = singles.tile([128, 4], F32)
        nc.sync.dma_start(out=bq_sb, in_=bq.rearrange("(b p) -> p b", p=128))
        bk_sb = singles.tile([128, 4], F32)
        nc.sync.dma_start(out=bk_sb, in_=bk.rearrange("(b p) -> p b", p=128))
        bv_b = singles.tile([128, D_HEAD], F32)
        nc.sync.dma_start(out=bv_b, in_=_bcast_ap(bv))
        gam_b = singles.tile([128, D_HEAD], F32)
        nc.sync.dma_start(out=gam_b, in_=_bcast_ap(gam))
        bet_b = singles.tile([128, D_HEAD], F32)
        nc.sync.dma_start(out=bet_b, in_=_bcast_ap(bet))
        eps_sb = singles.tile([128, 1], F32)
        nc.vector.memset(eps_sb, LN_EPS)

        gp_chain = [None]  # chain all gpsimd desc/trigger instrs across reps

        def chain(binst):
            if gp_chain[0] is not None:
                add_dep_helper(binst.ins, gp_chain[0].ins, reason="swdge order")
            gp_chain[0] = binst

        def emit_rep(R):
            loc_free = LOC_PER_REP * R  # all of rep R-1's sends have left
            qT = [
                resident.tile([128, N], WD, name=f"qT{f}", tag=f"qT{f}")
                for f in range(4)
            ]
            kT = [
                resident.tile([128, N], WD, name=f"kT{f}", tag=f"kT{f}")
                for f in range(4)
            ]
            v_sb = [
                resident.tile([128, D_HEAD], F32, name=f"v{t}", tag=f"v{t}")
                for t in range(4)
            ]
            sc_own = [
                comm.tile([128, N], F32, name=f"sco{t}", tag=f"sco{t}")
                for t in range(4)
            ]
            sc_send = comm.tile([128, 4 * N], F32, name="sc_send", tag="sc_send")
            sc_recv = comm.tile([128, 4 * N], F32, name="sc_recv", tag="sc_recv")
            st = comm.tile([128, 4, 2], F32, name="st", tag="st")
            st_r2 = comm.tile([128, 4, 2], F32, name="st_r2", tag="st_r2")
            st12 = comm.tile([128, 4, 2], F32, name="st12", tag="st12")
            st_r4 = comm.tile([128, 4, 2], F32, name="st_r4", tag="st_r4")
            tot = comm.tile([128, 4, 2], F32, name="tot", tag="tot")

            def projection(act2, w2, bias_fn):
                psums = [
                    [
                        ps.tile([128, 512], F32, name=f"pp{f}_{j}", tag=f"pp{f}_{j}")
                        for j in range(2)
                    ]
                    for f in range(4)
                ]
                for k4 in range(K_TILES // KB):
                    a_t = streams.tile([128, KB, N], WD, name="a_t", tag="a_t")
                    w_t = streams.tile([128, KB, 512], WD, name="w_t", tag="w_t")
                    if k4 == 0:
                        # split the first chunk's DMA so the PE can start on
                        # sub-chunk 0 ~3 us earlier
                        nc.sync.dma_start(out=a_t[:, 0:1, :], in_=act2[:, 0:1, :])
                        nc.sync.dma_start(out=a_t[:, 1:KB, :], in_=act2[:, 1:KB, :])
                        nc.scalar.dma_start(out=w_t[:, 0:1, :], in_=w2[:, 0:1, :])
                        nc.scalar.dma_start(out=w_t[:, 1:KB, :], in_=w2[:, 1:KB, :])
                    else:
                        nc.sync.dma_start(
                            out=a_t, in_=act2[:, KB * k4 : KB * k4 + KB, :]
                        )
                        nc.scalar.dma_start(
                            out=w_t, in_=w2[:, KB * k4 : KB * k4 + KB, :]
                        )
                    for a in range(KB):
                        first = k4 == 0 and a == 0
                        last = k4 == K_TILES // KB - 1 and a == KB - 1
                        for f in range(4):
                            for j in range(2):
                                nc.tensor.matmul(
                                    psums[f][j][:],
                                    w_t[:, a, f * 128 : (f + 1) * 128],
                                    a_t[:, a, j * 512 : (j + 1) * 512],
                                    start=first,
                                    stop=last,
                                )
                for f in range(4):
                    for j in range(2):
                        bias_fn(f, j, psums[f][j])

            # ---- Q phase ----
            projection(
                n1t_2,
                wq_2,
                lambda f, j, p: nc.vector.tensor_scalar(
                    out=qT[f][:, j * 512 : (j + 1) * 512],
                    in0=p[:],
                    scalar1=bq_sb[:, f : f + 1],
                    scalar2=None,
                    op0=ALU.add,
                ),
            )
            # ---- K phase ----
            projection(
                n2t_2,
                wk_2,
                lambda f, j, p: nc.vector.tensor_scalar(
                    out=kT[f][:, j * 512 : (j + 1) * 512],
                    in0=p[:],
                    scalar1=bk_sb[:, f : f + 1],
                    scalar2=None,
                    op0=ALU.add,
                ),
            )

            # ---- scores: pair rows (local nb 4..7) first -> sc_send -> rdma;
            #      own rows (nb 0..3) -> sc_own ----
            for nb in range(4, 8):
                t = nb - 4
                for mh in range(2):
                    sc_ps = ps.tile(
                        [128, 512], F32, name=f"sps{nb}_{mh}", tag=f"pp{nb % 4}_{mh}"
                    )
                    for ft in range(4):
                        nc.tensor.matmul(
                            sc_ps[:],
                            qT[ft][:, nb * 128 : (nb + 1) * 128],
                            kT[ft][:, mh * 512 : (mh + 1) * 512],
                            start=(ft == 0),
                            stop=(ft == 3),
                        )
                    cp = nc.vector.tensor_copy(
                        out=sc_send[:, t * N + mh * 512 : t * N + (mh + 1) * 512],
                        in_=sc_ps[:],
                    )
                    want_wait(cp, loc, loc_free)
                if local_comm:
                    nc.vector.tensor_copy(
                        out=sc_recv[:, t * N : (t + 1) * N],
                        in_=sc_send[:, t * N : (t + 1) * N],
                    )
                else:
                    b = nc.gpsimd.remote_dma_broadcast(
                        sc_recv[:, t * N : (t + 1) * N],
                        sc_send[:, t * N : (t + 1) * N],
                        sc_dat,
                        loc,
                        rdests=D1,
                    )
                    chain(b)
            if not local_comm:
                t1 = nc.gpsimd.trigger_dma(count=None)
                chain(t1)
                want_wait(t1, sc_ack, 16 * R)

            for nb in range(4):
                for mh in range(2):
                    sc_ps = ps.tile(
                        [128, 512], F32, name=f"sps{nb}_{mh}", tag=f"pp{nb % 4}_{mh}"
                    )
                    for ft in range(4):
                        nc.tensor.matmul(
                            sc_ps[:],
                            qT[ft][:, nb * 128 : (nb + 1) * 128],
                            kT[ft][:, mh * 512 : (mh + 1) * 512],
                            start=(ft == 0),
                            stop=(ft == 3),
                        )
                    nc.vector.tensor_copy(
                        out=sc_own[nb][:, mh * 512 : (mh + 1) * 512], in_=sc_ps[:]
                    )

            # ---- V phase: stationary n2v blocks, moving full wv ----
            vps = [
                [
                    ps.tile([128, 512], F32, name=f"vp{t}_{j}", tag=f"pp{t}_{j}")
                    for j in range(2)
                ]
                for t in range(4)
            ]
            for k4 in range(K_TILES // KB):
                nv_t = streams.tile([128, KB, 512], WD, name="nv_t", tag="nv_t")
                nc.scalar.dma_start(out=nv_t, in_=n2v_2[:, KB * k4 : KB * k4 + KB, :])
                wv_t = streams.tile([128, KB, D_HEAD], WD, name="wv_t", tag="wv_t")
                nc.sync.dma_start(out=wv_t, in_=wv_2[:, KB * k4 : KB * k4 + KB, :])
                for a in range(KB):
                    first = k4 == 0 and a == 0
                    last = k4 == K_TILES // KB - 1 and a == KB - 1
                    for t in range(4):
                        for j in range(2):
                            nc.tensor.matmul(
                                vps[t][j][:],
                                nv_t[:, a, t * 128 : (t + 1) * 128],
                                wv_t[:, a, j * 512 : (j + 1) * 512],
                                start=first,
                                stop=last,
                            )

            # ---- r_pre = own + recv partials (overlaps V matmuls) ----
            r_pre = [
                fin.tile([128, N], F32, name=f"rp{t}", tag=f"rp{t}") for t in range(4)
            ]
            adds = []
            for t in range(4):
                ad = nc.vector.tensor_add(
                    out=r_pre[t][:],
                    in0=sc_own[t][:],
                    in1=sc_recv[:, t * N : (t + 1) * N],
                )
                want_wait(ad, sc_dat, 64 * (R + 1))
                adds.append(ad)

            for t in range(4):
                for j in range(2):
                    nc.vector.tensor_add(
                        out=v_sb[t][:, j * 512 : (j + 1) * 512],
                        in0=vps[t][j][:],
                        in1=bv_b[:, j * 512 : (j + 1) * 512],
                    )

            # ---- r = r_pre .* v; LN stat partials ----
            sq_scr = fin.tile([128, N], F32, name="sq_scr", tag="sq_scr")
            st_writers = []
            for t in range(4):
                nc.vector.tensor_mul(out=r_pre[t][:], in0=r_pre[t][:], in1=v_sb[t][:])
                rd = nc.vector.tensor_reduce(
                    out=st[:, t, 0:1], in_=r_pre[t][:],
                    axis=mybir.AxisListType.X, op=ALU.add,
                )
                sqa = nc.scalar.activation(
                    out=sq_scr[:], in_=r_pre[t][:], func=ACT_FN.Square,
                    accum_out=st[:, t, 1:2],
                )
                want_wait(rd, loc, loc_free)
                want_wait(sqa, loc, loc_free)
                st_writers.append((rd, sqa))

            # ---- stats exchange ----
            if stats_ag:
                ag_in = dram.tile([512, 2], F32, name="ag_in", tag="ag_in", bufs=2)
                ag_out = dram.tile([2048, 2], F32, name="ag_out", tag="ag_out", bufs=2)
                nc.gpsimd.dma_start(
                    out=ag_in[:].rearrange("(t p) c -> p t c", p=128), in_=st[:]
                )
                nc.gpsimd.collective_compute(
                    "AllGather",
                    ALU.bypass,
                    replica_groups=[[0, 2, 4, 6], [1, 3, 5, 7]],
                    ins=[ag_in[:].opt()],
                    outs=[ag_out[:].opt()],
                )
                agq = fin.tile([128, 4, 4, 2], F32, name="agq", tag="agq")
                nc.gpsimd.dma_start(
                    out=agq, in_=ag_out[:].rearrange("(q t p) c -> p q t c", p=128)
                )
                t01 = fin.tile([128, 4, 2], F32, name="t01", tag="t01")
                nc.vector.tensor_add(out=t01[:], in0=agq[:, 0], in1=agq[:, 1])
                t23 = fin.tile([128, 4, 2], F32, name="t23", tag="t23")
                nc.vector.tensor_add(out=t23[:], in0=agq[:, 2], in1=agq[:, 3])
                atot = nc.vector.tensor_add(out=tot[:], in0=t01[:], in1=t23[:])
                a12 = atot  # ack bookkeeping below reuses these names
            elif local_comm:
                nc.vector.tensor_copy(out=st_r2[:], in_=st[:])
                a12 = None
            else:
                if not local_comm:
                    b2 = nc.gpsimd.remote_dma_broadcast(st_r2[:], st[:], q2_dat, loc, rdests=D2)
                    chain(b2)
                    t2 = nc.gpsimd.trigger_dma(count=None)
                    chain(t2)
                    want_wait(t2, q2_ack, 16 * R)
                a12 = nc.vector.tensor_add(out=st12[:], in0=st[:], in1=st_r2[:])
                want_wait(a12, q2_dat, 16 * (R + 1))
                want_wait(a12, loc, loc_free)

                # ---- stats round 2: other die (flip-immune) ----
                if local_comm:
                    nc.vector.tensor_copy(out=st_r4[:], in_=st12[:])
                else:
                    b4 = nc.gpsimd.remote_dma_broadcast(st_r4[:], st12[:], q4_dat, loc, rdests=D4)
                    chain(b4)
                    t3 = nc.gpsimd.trigger_dma(count=None)
                    chain(t3)
                    want_wait(t3, q4_ack, 8 * R)
                atot = nc.vector.tensor_add(out=tot[:], in0=st12[:], in1=st_r4[:])
                want_wait(atot, q4_dat, 8 * (R + 1))

            # ---- acks (consumed-my-recv-buffers) for next rep ----
            if not local_comm:
                k1 = nc.gpsimd.remote_sem_update_broadcast(sc_ack, loc, rdests=D1)
                for ad in adds:
                    add_dep_helper(k1.ins, ad.ins, reason="sc_recv consumed")
                chain(k1)
                if not stats_ag:
                    k2 = nc.gpsimd.remote_sem_update_broadcast(q2_ack, loc, rdests=D2)
                    add_dep_helper(k2.ins, a12.ins, reason="st_r2 consumed")
                    chain(k2)
                    k4i = nc.gpsimd.remote_sem_update_broadcast(q4_ack, loc, rdests=D4)
                    add_dep_helper(k4i.ins, atot.ins, reason="st_r4 consumed")
                    chain(k4i)
                t4 = nc.gpsimd.trigger_dma(count=None)
                chain(t4)

            # ---- normalize + out ----
            inv_h = 1.0 / float(H_DIM)
            for t in range(4):
                mu_t = fin.tile([128, 1], F32, name=f"mu{t}", tag=f"mu{t}")
                nc.vector.tensor_scalar_mul(out=mu_t, in0=tot[:, t, 0:1], scalar1=inv_h)
                msq_t = fin.tile([128, 1], F32, name=f"msq{t}", tag=f"msq{t}")
                nc.vector.tensor_mul(out=msq_t, in0=mu_t, in1=mu_t)
                var_t = fin.tile([128, 1], F32, name=f"var{t}", tag=f"var{t}")
                nc.vector.tensor_scalar(
                    out=var_t,
                    in0=tot[:, t, 1:2],
                    scalar1=inv_h,
                    scalar2=msq_t[:, 0:1],
                    op0=ALU.mult,
                    op1=ALU.subtract,
                )
                nc.scalar.activation(
                    out=var_t, in_=var_t, func=ACT_FN.Sqrt, bias=eps_sb[:], scale=1.0
                )
                nc.vector.reciprocal(out=var_t, in_=var_t)
                o_t = fin.tile([128, N], F32, name="o_t", tag="o_t", bufs=2)[:]
                nc.vector.tensor_scalar(
                    out=o_t,
                    in0=r_pre[t][:],
                    scalar1=mu_t[:, 0:1],
                    scalar2=var_t[:, 0:1],
                    op0=ALU.subtract,
                    op1=ALU.mult,
                )
                nc.vector.tensor_mul(out=o_t, in0=o_t, in1=gam_b[:])
                nc.vector.tensor_add(out=o_t, in0=o_t, in1=bet_b[:])
                nc.sync.dma_start(out=out[t * 128 : (t + 1) * 128, :], in_=o_t)

        for rep in range(reps):
            emit_rep(rep)

    # encode runtime-only cross-core waits (post-scheduling)
    for binst, sem, val in post_waits:
        ins = binst.ins
        si = ins.sync_info
        waits = list(si.on_wait) if si else []
        ups = list(si.on_update) if si else []
        waits.append(
            mybir.SyncWait(
                sync_type="semaphore", id=sem.num, ant_name=sem.name,
                wait_mode="sem-ge-imm", wait_value=val,
            )
        )
        ins.sync_info = mybir.SyncInfo(on_wait=waits, on_update=ups)

    nc.compile()
    return nc


_NC = None


def _get_program():
    global _NC
    if _NC is None:
        _NC = build_program()
    return _NC


def make_in_maps(node1, node2, Wq, bq, Wk, bk, Wv, bv, gamma, beta, wire: str = WIRE):
    import ml_dtypes

    f32 = np.float32
    wd = ml_dtypes.bfloat16 if wire == "bf16" else np.float32
    n1t = np.ascontiguousarray(np.asarray(node1).T, dtype=f32)
    n2t = np.ascontiguousarray(np.asarray(node2).T, dtype=f32)
    n1t_w = n1t.astype(wd)
    n2t_w = n2t.astype(wd)
    # odd cores see node1's row halves swapped -> local score rows 0:511
    # are always the core's own rows (SPMD-symmetric send/keep split)
    n1t_sw = np.ascontiguousarray(
        np.concatenate([n1t_w[:, 512:], n1t_w[:, :512]], axis=1)
    )
    in_maps = []
    for c in range(N_CORES):
        h, s = c // 2, c % 2
        fsl = slice(s * 512, (s + 1) * 512)
        nsl = slice(s * 512, (s + 1) * 512)
        in_maps.append(
            {
                "n1t": n1t_w if s == 0 else n1t_sw,
                "n2t": n2t_w,
                "n2v": np.ascontiguousarray(n2t_w[:, nsl]),
                "wq": np.ascontiguousarray(Wq[h][:, fsl] * SCALE).astype(wd),
                "wk": np.ascontiguousarray(Wk[h][:, fsl]).astype(wd),
                "wv": np.ascontiguousarray(Wv[h]).astype(wd),
                "bq": np.ascontiguousarray(bq[h][fsl] * SCALE, dtype=f32),
                "bk": np.ascontiguousarray(bk[h][fsl], dtype=f32),
                "bv": np.ascontiguousarray(bv[h], dtype=f32),
                "gam": np.ascontiguousarray(gamma[h * 1024 : (h + 1) * 1024], dtype=f32),
                "bet": np.ascontiguousarray(beta[h * 1024 : (h + 1) * 1024], dtype=f32),
            }
        )
    return in_maps


def assemble(results):
    out = np.empty((N, H_DIM), np.float32)
    for c in range(N_CORES):
        h, s = c // 2, c % 2
        out[s * 512 : (s + 1) * 512, h * 1024 : (h + 1) * 1024] = results[c]["out"]
    return out


def kernel(node1, node2, Wq, bq, Wk, bk, Wv, bv, gamma, beta):
    nc = _get_program()
    in_maps = make_in_maps(node1, node2, Wq, bq, Wk, bk, Wv, bv, gamma, beta)
    res = run_bass_kernel_spmd(nc, in_maps, list(range(N_CORES)))
    return assemble(res.results)
